# revision 2
# baseline (speedup 1.0000x reference)
"""AutoRound/GPTQ int4 linear on 8 Trainium2 NeuronCores.

y = x @ dequant(qweight, qzeros, scales), computed in bf16 like the torch
module: deq = (w_int4 - zeros[g]) * scales[g] in fp32, cast to bf16;
y = bf16_matmul(x.bf16, deq.bf16) with fp32 accumulation, output cast
back to fp32.

Sharding: 8 cores = 4-way tensor-parallel on out_features (1024 each)
x 2-way data-parallel on tokens (4096 each). Each core dequantizes its
weight slice on-chip and computes y_part^T = deq_slice^T-style matmul
producing [1024 out, 4096 tok] bf16; the host reassembles.

Device-side layout trick: the contraction (in_features) index is
interleaved so that SBUF k-chunk `cc = blk*8 + j` holds k = blk*1024 +
8*p + j at partition p. Then nibble j of packed qweight row p (of the
block's 128 rows) is exactly the weight for partition p of chunk cc, so
the int4 unpack is one fused shift+mask tensor_scalar op per chunk with
a *constant* shift, and the 128-row group structure (GROUP=128) makes
zeros/scales constant per 16-partition band, loaded with broadcast DMAs.
The host feeds x^T with rows permuted the same way, so the matmul
contraction is consistent.
"""

import numpy as np
import ml_dtypes

PACK = 8
IN_F = 4096
OUT_F = 4096
GROUP = 128
B, S = 4, 2048
T_TOTAL = B * S  # 8192

N_CORES = 8
TP = 4  # out_feature shards
DP = 2  # token shards
NO = OUT_F // TP  # 1024 out features per core
TP_T = T_TOTAL // DP  # 4096 tokens per core
NT = 512  # token tile (matmul moving free dim / one PSUM bank)
KB = IN_F // 1024  # k blocks of 1024 (8 chunks of 128 each)


def build_nc(no=NO, t=TP_T, nt=NT, kblocks=KB):
    import concourse.bacc as bacc
    import concourse.mybir as mybir
    from concourse.tile import TileContext

    dt = mybir.dt
    alu = mybir.AluOpType
    n_chunks = kblocks * 8

    nc = bacc.Bacc("TRN2", target_bir_lowering=False, debug=False)

    xt_d = nc.dram_tensor("xt", [n_chunks * 128, t], dt.float32, kind="ExternalInput")
    qw_d = nc.dram_tensor("qw", [kblocks * 128, no], dt.int32, kind="ExternalInput")
    qz_d = nc.dram_tensor("qz", [kblocks * 8, no // 8], dt.int32, kind="ExternalInput")
    sc_d = nc.dram_tensor("sc", [kblocks * 8, no], dt.float16, kind="ExternalInput")
    y_d = nc.dram_tensor("y", [no, t], dt.bfloat16, kind="ExternalOutput")

    with TileContext(nc) as tc:
        with (
            tc.tile_pool(name="wd", bufs=1) as wd_pool,
            tc.tile_pool(name="qw", bufs=2) as qw_pool,
            tc.tile_pool(name="qzbc", bufs=2) as qzbc_pool,
            tc.tile_pool(name="sbc", bufs=2) as sbc_pool,
            tc.tile_pool(name="zf", bufs=2) as zf_pool,
            tc.tile_pool(name="wi", bufs=3) as wi_pool,
            tc.tile_pool(name="xf", bufs=6) as xf_pool,
            tc.tile_pool(name="xbf", bufs=2) as xbf_pool,
            tc.tile_pool(name="ps", bufs=8, space="PSUM") as ps_pool,
            tc.tile_pool(name="yo", bufs=4) as yo_pool,
        ):
            # ---- dequantize weight slice into persistent Wd [128, n_chunks*no] bf16
            wd = wd_pool.tile([128, n_chunks * no], dt.bfloat16)
            for blk in range(kblocks):
                qw_sb = qw_pool.tile([128, no], dt.int32)
                nc.sync.dma_start(
                    out=qw_sb[:], in_=qw_d[blk * 128 : (blk + 1) * 128, :]
                )
                qz_bc = qzbc_pool.tile([128, no // 8], dt.int32)
                sbc = sbc_pool.tile([128, no], dt.float16)
                for g in range(8):
                    nc.sync.dma_start(
                        out=qz_bc[g * 16 : (g + 1) * 16, :],
                        in_=qz_d[blk * 8 + g][None, :].broadcast_to([16, no // 8]),
                    )
                    nc.sync.dma_start(
                        out=sbc[g * 16 : (g + 1) * 16, :],
                        in_=sc_d[blk * 8 + g][None, :].broadcast_to([16, no]),
                    )
                zf = zf_pool.tile([128, no], dt.int32)
                zf_v = zf[:].rearrange("p (q e) -> p q e", e=8)
                for jz in range(8):
                    nc.vector.tensor_scalar(
                        out=zf_v[:, :, jz],
                        in0=qz_bc[:],
                        scalar1=4 * jz,
                        scalar2=15,
                        op0=alu.logical_shift_right,
                        op1=alu.bitwise_and,
                    )
                for j in range(8):
                    cc = blk * 8 + j
                    wi = wi_pool.tile([128, no], dt.int32, tag="wi_i")
                    nc.vector.tensor_scalar(
                        out=wi[:],
                        in0=qw_sb[:],
                        scalar1=4 * j,
                        scalar2=15,
                        op0=alu.logical_shift_right,
                        op1=alu.bitwise_and,
                    )
                    wb = wi_pool.tile([128, no], dt.bfloat16, tag="wi_b")
                    nc.vector.tensor_sub(out=wb[:], in0=wi[:], in1=zf[:])
                    nc.vector.tensor_mul(
                        out=wd[:, cc * no : (cc + 1) * no], in0=wb[:], in1=sbc[:]
                    )

            # ---- stream token tiles: load fp32, cast bf16, matmul, store
            for tt in range(t // nt):
                xbf = xbf_pool.tile([128, n_chunks * nt], dt.bfloat16)
                for k in range(n_chunks):
                    xf = xf_pool.tile([128, nt], dt.float32)
                    nc.sync.dma_start(
                        out=xf[:],
                        in_=xt_d[k * 128 : (k + 1) * 128, tt * nt : (tt + 1) * nt],
                    )
                    eng = nc.vector if (k % 2 == 0) else nc.scalar
                    if eng is nc.vector:
                        eng.tensor_copy(out=xbf[:, k * nt : (k + 1) * nt], in_=xf[:])
                    else:
                        eng.copy(out=xbf[:, k * nt : (k + 1) * nt], in_=xf[:])
                for os_ in range(no // 128):
                    ps = ps_pool.tile([128, nt], dt.float32)
                    for k in range(n_chunks):
                        nc.tensor.matmul(
                            out=ps[:],
                            lhsT=wd[:, k * no + os_ * 128 : k * no + os_ * 128 + 128],
                            rhs=xbf[:, k * nt : (k + 1) * nt],
                            start=(k == 0),
                            stop=(k == n_chunks - 1),
                        )
                    yo = yo_pool.tile([128, nt], dt.bfloat16)
                    if os_ % 2 == 0:
                        nc.scalar.copy(out=yo[:], in_=ps[:])
                    else:
                        nc.vector.tensor_copy(out=yo[:], in_=ps[:])
                    nc.sync.dma_start(
                        out=y_d[os_ * 128 : (os_ + 1) * 128, tt * nt : (tt + 1) * nt],
                        in_=yo[:],
                    )
    nc.compile()
    return nc


def shard_inputs(x, qweight, qzeros, scales, no=NO, t=TP_T):
    """Host-side sharding + the k-interleave layout for x^T."""
    x2 = np.asarray(x, dtype=np.float32).reshape(T_TOTAL, IN_F)
    qweight = np.asarray(qweight)
    qzeros = np.asarray(qzeros)
    scales = np.asarray(scales)

    # xr[blk*1024 + j*128 + p, tok] = x2[tok, blk*1024 + 8p + j]
    xv = x2.reshape(T_TOTAL, IN_F // 1024, 128, 8)  # [tok, blk, p, j]
    xt_shards = []
    for r in range(DP):
        sl = xv[r * t : (r + 1) * t]  # [t, blk, p, j]
        xr = np.ascontiguousarray(sl.transpose(1, 3, 2, 0)).reshape(IN_F, t)
        xt_shards.append(xr)

    in_maps = []
    for core in range(N_CORES):
        r, c = divmod(core, TP)
        in_maps.append(
            {
                "xt": xt_shards[r],
                "qw": np.ascontiguousarray(qweight[:, c * no : (c + 1) * no]),
                "qz": np.ascontiguousarray(
                    qzeros[:, c * (no // 8) : (c + 1) * (no // 8)]
                ),
                "sc": np.ascontiguousarray(scales[:, c * no : (c + 1) * no]),
            }
        )
    return in_maps


def assemble_output(results, no=NO, t=TP_T):
    y = np.empty((T_TOTAL, OUT_F), dtype=np.float32)
    for core in range(N_CORES):
        r, c = divmod(core, TP)
        yp = np.asarray(results[core]["y"])  # [no, t] bf16
        y[r * t : (r + 1) * t, c * no : (c + 1) * no] = yp.T.astype(np.float32)
    return y.reshape(B, S, OUT_F)


_NC_CACHE = {}


def run(x, qweight, qzeros, scales, trace=False, tmpdir=None):
    from concourse.bass_utils import run_bass_kernel_spmd

    if "nc" not in _NC_CACHE:
        _NC_CACHE["nc"] = build_nc()
    nc = _NC_CACHE["nc"]
    in_maps = shard_inputs(x, qweight, qzeros, scales)
    res = run_bass_kernel_spmd(
        nc, in_maps, list(range(N_CORES)), trace=trace, tmpdir=tmpdir
    )
    return assemble_output(res.results), res


def kernel(x, qweight, qzeros, scales):
    y, _ = run(x, qweight, qzeros, scales)
    return y


# revision 29
# speedup vs baseline: 1.1164x; 1.1164x over previous
"""AutoRound/GPTQ int4 linear on 8 Trainium2 NeuronCores.

y = x @ dequant(qweight, qzeros, scales), computed in bf16 like the torch
module: deq = (w_int4 - zeros[g]) * scales[g] in fp32, cast to bf16;
y = bf16_matmul(x.bf16, deq.bf16) with fp32 accumulation, output cast
back to fp32.

Sharding: 8 cores = 4-way tensor-parallel on out_features (1024 each)
x 2-way data-parallel on tokens (4096 each). Each core dequantizes its
weight slice on-chip and computes y_part^T = deq_slice^T-style matmul
producing [1024 out, 4096 tok] bf16; the host reassembles.

Device-side layout trick: the contraction (in_features) index is
interleaved so that SBUF k-chunk `cc = blk*8 + j` holds k = blk*1024 +
8*p + j at partition p. Then nibble j of packed qweight row p (of the
block's 128 rows) is exactly the weight for partition p of chunk cc, so
the int4 unpack is one fused shift+mask tensor_scalar op per chunk with
a *constant* shift, and the 128-row group structure (GROUP=128) makes
zeros/scales constant per 16-partition band, loaded with broadcast DMAs.
The host feeds x^T with rows permuted the same way, so the matmul
contraction is consistent.
"""

import numpy as np
import ml_dtypes

PACK = 8
IN_F = 4096
OUT_F = 4096
GROUP = 128
B, S = 4, 2048
T_TOTAL = B * S  # 8192

N_CORES = 8
TP = 4  # out_feature shards
DP = 2  # token shards
NO = OUT_F // TP  # 1024 out features per core
TP_T = T_TOTAL // DP  # 4096 tokens per core
NT = 512  # token tile (matmul moving free dim / one PSUM bank)
KB = IN_F // 1024  # k blocks of 1024 (8 chunks of 128 each)


def build_nc(no=NO, t=TP_T, nt=NT, kblocks=KB):
    import concourse.bacc as bacc
    import concourse.mybir as mybir
    from concourse.tile import TileContext

    dt = mybir.dt
    alu = mybir.AluOpType
    n_chunks = kblocks * 8

    nc = bacc.Bacc("TRN2", target_bir_lowering=False, debug=False)

    xt_d = nc.dram_tensor("xt", [n_chunks * 128, t], dt.float32, kind="ExternalInput")
    # low/high int16 halves of the packed int32 qweight/qzeros (host-split):
    # nibbles j=0..3 live in the low half, j=4..7 in the high half.
    qwl_d = nc.dram_tensor("qwl", [kblocks * 128, no], dt.int16, kind="ExternalInput")
    qwh_d = nc.dram_tensor("qwh", [kblocks * 128, no], dt.int16, kind="ExternalInput")
    # zeros/scales with group rows pre-replicated x16 on host (row p = group p//16)
    qzl_d = nc.dram_tensor(
        "qzl", [kblocks * 128, no // 8], dt.int16, kind="ExternalInput"
    )
    qzh_d = nc.dram_tensor(
        "qzh", [kblocks * 128, no // 8], dt.int16, kind="ExternalInput"
    )
    sc_d = nc.dram_tensor("sc", [kblocks * 128, no], dt.float16, kind="ExternalInput")
    y_d = nc.dram_tensor("y", [no, t], dt.bfloat16, kind="ExternalOutput")

    with TileContext(nc) as tc:
        with (
            tc.tile_pool(name="wd", bufs=1) as wd_pool,
            tc.tile_pool(name="qw", bufs=2) as qw_pool,
            tc.tile_pool(name="qzbc", bufs=2) as qzbc_pool,
            tc.tile_pool(name="sbc", bufs=2) as sbc_pool,
            tc.tile_pool(name="zf", bufs=2) as zf_pool,
            tc.tile_pool(name="wi", bufs=5) as wi_pool,
            tc.tile_pool(name="xbf", bufs=2) as xbf_pool,
            tc.tile_pool(name="ps", bufs=8, space="PSUM") as ps_pool,
            tc.tile_pool(name="yo", bufs=4) as yo_pool,
        ):
            # ---- PE warm-up: dummy matmuls on a memset tile so the HAM
            # clock-gate reaches 2.4 GHz before the real stream starts.
            warm = qw_pool.tile([128, nt], dt.bfloat16, tag="warm")
            nc.vector.memset(warm[:], 0.0)
            ps_w = ps_pool.tile([128, nt], dt.float32, tag="ps")
            for _ in range(40):
                nc.tensor.matmul(
                    out=ps_w[:],
                    lhsT=warm[:, 0:128],
                    rhs=warm[:],
                    start=True,
                    stop=True,
                )

            # ---- dequantize weight slice into 32 per-chunk tiles [128, no] bf16
            wd_tiles = [None] * n_chunks
            qw_sbs = []
            zf_tiles = [None] * kblocks
            sbc_tiles = [None] * kblocks

            def load_block(blk):
                qwl_sb = qw_pool.tile([128, no], dt.int16, tag=f"qwl{blk % 2}")
                qwh_sb = qw_pool.tile([128, no], dt.int16, tag=f"qwh{blk % 2}")
                qw_sbs.append((qwl_sb, qwh_sb))
                qzl_bc = qzbc_pool.tile([128, no // 8], dt.int16, tag=f"qzl{blk % 2}")
                qzh_bc = qzbc_pool.tile([128, no // 8], dt.int16, tag=f"qzh{blk % 2}")
                sbc = sbc_pool.tile([128, no], dt.float16, tag=f"sbc{blk % 2}")
                nc.sync.dma_start(
                    out=qzl_bc[:], in_=qzl_d[blk * 128 : (blk + 1) * 128, :]
                )
                nc.sync.dma_start(
                    out=qzh_bc[:], in_=qzh_d[blk * 128 : (blk + 1) * 128, :]
                )
                nc.sync.dma_start(
                    out=qwl_sb[:], in_=qwl_d[blk * 128 : (blk + 1) * 128, :]
                )
                nc.sync.dma_start(
                    out=qwh_sb[:], in_=qwh_d[blk * 128 : (blk + 1) * 128, :]
                )
                nc.scalar.dma_start(
                    out=sbc[:], in_=sc_d[blk * 128 : (blk + 1) * 128, :]
                )
                sbc_tiles[blk] = sbc
                return qzl_bc, qzh_bc

            def unpack_zeros_op(blk, qzl_bc, qzh_bc, jz):
                if jz == 0:
                    zf_tiles[blk] = zf_pool.tile(
                        [128, no], dt.int16, tag=f"zf{blk % 2}", name=f"zf{blk}"
                    )
                zf_v = zf_tiles[blk][:].rearrange("p (q e) -> p q e", e=8)
                nc.vector.tensor_scalar(
                    out=zf_v[:, :, jz],
                    in0=(qzl_bc if jz < 4 else qzh_bc)[:],
                    scalar1=4 * (jz % 4),
                    scalar2=15,
                    op0=alu.logical_shift_right,
                    op1=alu.bitwise_and,
                )

            def unpack_zeros(blk, qzl_bc, qzh_bc):
                for jz in range(8):
                    unpack_zeros_op(blk, qzl_bc, qzh_bc, jz)

            qz_next = load_block(0)
            unpack_zeros(0, *qz_next)
            for blk in range(kblocks):
                qwl_sb, qwh_sb = qw_sbs[blk]
                for j in range(8):
                    cc = blk * 8 + j
                    wi = wi_pool.tile([128, no], dt.int16, tag="wi_i")
                    nc.vector.tensor_scalar(
                        out=wi[:],
                        in0=(qwl_sb if j < 4 else qwh_sb)[:],
                        scalar1=4 * (j % 4),
                        scalar2=15,
                        op0=alu.logical_shift_right,
                        op1=alu.bitwise_and,
                    )
                    wb = wi_pool.tile([128, no], dt.bfloat16, tag="wi_b")
                    nc.vector.tensor_sub(out=wb[:], in0=wi[:], in1=zf_tiles[blk][:])
                    wdc = wd_pool.tile([128, no], dt.bfloat16, tag=f"wd{cc}")
                    nc.vector.tensor_mul(out=wdc[:], in0=wb[:], in1=sbc_tiles[blk][:])
                    wd_tiles[cc] = wdc
                    if j == 0 and blk + 1 < kblocks:
                        # prefetch next block's inputs early
                        qz_next = load_block(blk + 1)
                    if blk + 1 < kblocks:
                        # spread the next block's 8 zero-unpack ops one per
                        # chunk so chunk production never pauses in a lump
                        unpack_zeros_op(blk + 1, *qz_next, jz=j)

            # ---- stream token tiles: cast-DMA to bf16, matmul, store
            for tt in range(t // nt):
                xbf_t = []
                for k in range(n_chunks):
                    xb = xbf_pool.tile(
                        [128, nt], dt.bfloat16, tag=f"xb{k}", name=f"xb{k}"
                    )
                    nc.gpsimd.dma_start(
                        out=xb[:],
                        in_=xt_d[k * 128 : (k + 1) * 128, tt * nt : (tt + 1) * nt],
                    )
                    xbf_t.append(xb)
                for os_ in range(no // 128):
                    ps = ps_pool.tile([128, nt], dt.float32, tag="ps", name="ps")
                    ks = [(os_ * 4 + i) % n_chunks for i in range(n_chunks)]
                    for i, k in enumerate(ks):
                        nc.tensor.matmul(
                            out=ps[:],
                            lhsT=wd_tiles[k][:, os_ * 128 : (os_ + 1) * 128],
                            rhs=xbf_t[k][:],
                            start=(i == 0),
                            stop=(i == n_chunks - 1),
                        )
                    yo = yo_pool.tile([128, nt], dt.bfloat16, name="yo")
                    if os_ % 2 == 0:
                        nc.scalar.copy(out=yo[:], in_=ps[:])
                    else:
                        nc.vector.tensor_copy(out=yo[:], in_=ps[:])
                    nc.sync.dma_start(
                        out=y_d[os_ * 128 : (os_ + 1) * 128, tt * nt : (tt + 1) * nt],
                        in_=yo[:],
                    )
    nc.compile()
    return nc


def shard_inputs(x, qweight, qzeros, scales, no=NO, t=TP_T):
    """Host-side sharding + the k-interleave layout for x^T."""
    x2 = np.asarray(x, dtype=np.float32).reshape(T_TOTAL, IN_F)
    qweight = np.asarray(qweight)
    qzeros = np.asarray(qzeros)
    scales = np.asarray(scales)

    # xr[blk*1024 + j*128 + p, tok] = x2[tok, blk*1024 + 8p + j]
    xv = x2.reshape(T_TOTAL, IN_F // 1024, 128, 8)  # [tok, blk, p, j]
    xt_shards = []
    for r in range(DP):
        sl = xv[r * t : (r + 1) * t]  # [t, blk, p, j]
        xr = np.ascontiguousarray(sl.transpose(1, 3, 2, 0)).reshape(IN_F, t)
        xt_shards.append(xr)

    qw16 = qweight.view(np.int16).reshape(qweight.shape[0], qweight.shape[1], 2)
    qz16 = qzeros.view(np.int16).reshape(qzeros.shape[0], qzeros.shape[1], 2)
    in_maps = []
    for core in range(N_CORES):
        r, c = divmod(core, TP)
        qwc = qw16[:, c * no : (c + 1) * no]
        qzc = qz16[:, c * (no // 8) : (c + 1) * (no // 8)]
        in_maps.append(
            {
                "xt": xt_shards[r],
                "qwl": np.ascontiguousarray(qwc[:, :, 0]),
                "qwh": np.ascontiguousarray(qwc[:, :, 1]),
                "qzl": np.repeat(np.ascontiguousarray(qzc[:, :, 0]), 16, axis=0),
                "qzh": np.repeat(np.ascontiguousarray(qzc[:, :, 1]), 16, axis=0),
                "sc": np.repeat(scales[:, c * no : (c + 1) * no], 16, axis=0),
            }
        )
    return in_maps


def assemble_output(results, no=NO, t=TP_T):
    y = np.empty((T_TOTAL, OUT_F), dtype=np.float32)
    for core in range(N_CORES):
        r, c = divmod(core, TP)
        yp = np.asarray(results[core]["y"])  # [no, t] bf16
        y[r * t : (r + 1) * t, c * no : (c + 1) * no] = yp.T.astype(np.float32)
    return y.reshape(B, S, OUT_F)


_NC_CACHE = {}


def run(x, qweight, qzeros, scales, trace=False, tmpdir=None):
    from concourse.bass_utils import run_bass_kernel_spmd

    if "nc" not in _NC_CACHE:
        _NC_CACHE["nc"] = build_nc()
    nc = _NC_CACHE["nc"]
    in_maps = shard_inputs(x, qweight, qzeros, scales)
    res = run_bass_kernel_spmd(
        nc, in_maps, list(range(N_CORES)), trace=trace, tmpdir=tmpdir
    )
    return assemble_output(res.results), res


def kernel(x, qweight, qzeros, scales):
    y, _ = run(x, qweight, qzeros, scales)
    return y


# revision 30
# speedup vs baseline: 1.1385x; 1.0198x over previous
"""AutoRound/GPTQ int4 linear on 8 Trainium2 NeuronCores.

y = x @ dequant(qweight, qzeros, scales), computed in bf16 like the torch
module: deq = (w_int4 - zeros[g]) * scales[g] in fp32, cast to bf16;
y = bf16_matmul(x.bf16, deq.bf16) with fp32 accumulation, output cast
back to fp32.

Sharding: 8 cores = 4-way tensor-parallel on out_features (1024 each)
x 2-way data-parallel on tokens (4096 each). Each core dequantizes its
weight slice on-chip and computes y_part^T = deq_slice^T-style matmul
producing [1024 out, 4096 tok] bf16; the host reassembles.

Device-side layout trick: the contraction (in_features) index is
interleaved so that SBUF k-chunk `cc = blk*8 + j` holds k = blk*1024 +
8*p + j at partition p. Then nibble j of packed qweight row p (of the
block's 128 rows) is exactly the weight for partition p of chunk cc, so
the int4 unpack is one fused shift+mask tensor_scalar op per chunk with
a *constant* shift, and the 128-row group structure (GROUP=128) makes
zeros/scales constant per 16-partition band, loaded with broadcast DMAs.
The host feeds x^T with rows permuted the same way, so the matmul
contraction is consistent.
"""

import numpy as np
import ml_dtypes

PACK = 8
IN_F = 4096
OUT_F = 4096
GROUP = 128
B, S = 4, 2048
T_TOTAL = B * S  # 8192

N_CORES = 8
TP = 4  # out_feature shards
DP = 2  # token shards
NO = OUT_F // TP  # 1024 out features per core
TP_T = T_TOTAL // DP  # 4096 tokens per core
NT = 512  # token tile (matmul moving free dim / one PSUM bank)
KB = IN_F // 1024  # k blocks of 1024 (8 chunks of 128 each)


def build_nc(no=NO, t=TP_T, nt=NT, kblocks=KB):
    import concourse.bacc as bacc
    import concourse.mybir as mybir
    from concourse.tile import TileContext

    dt = mybir.dt
    alu = mybir.AluOpType
    n_chunks = kblocks * 8

    nc = bacc.Bacc("TRN2", target_bir_lowering=False, debug=False)

    xt_d = nc.dram_tensor("xt", [n_chunks * 128, t], dt.float32, kind="ExternalInput")
    # low/high int16 halves of the packed int32 qweight/qzeros (host-split):
    # nibbles j=0..3 live in the low half, j=4..7 in the high half.
    qwl_d = nc.dram_tensor("qwl", [kblocks * 128, no], dt.int16, kind="ExternalInput")
    qwh_d = nc.dram_tensor("qwh", [kblocks * 128, no], dt.int16, kind="ExternalInput")
    # zeros (host-unpacked int16) and scales, group rows pre-replicated x16
    # on host so row p of a block corresponds to group p//16
    zf_d = nc.dram_tensor("zf", [kblocks * 128, no], dt.int16, kind="ExternalInput")
    sc_d = nc.dram_tensor("sc", [kblocks * 128, no], dt.float16, kind="ExternalInput")
    y_d = nc.dram_tensor("y", [no, t], dt.bfloat16, kind="ExternalOutput")

    with TileContext(nc) as tc:
        with (
            tc.tile_pool(name="wd", bufs=1) as wd_pool,
            tc.tile_pool(name="qw", bufs=2) as qw_pool,
            tc.tile_pool(name="sbc", bufs=2) as sbc_pool,
            tc.tile_pool(name="zf", bufs=2) as zf_pool,
            tc.tile_pool(name="wi", bufs=5) as wi_pool,
            tc.tile_pool(name="xbf", bufs=2) as xbf_pool,
            tc.tile_pool(name="ps", bufs=8, space="PSUM") as ps_pool,
            tc.tile_pool(name="yo", bufs=4) as yo_pool,
        ):
            # ---- PE warm-up: dummy matmuls on a memset tile so the HAM
            # clock-gate reaches 2.4 GHz before the real stream starts.
            warm = qw_pool.tile([128, nt], dt.bfloat16, tag="warm")
            nc.vector.memset(warm[:], 0.0)
            ps_w = ps_pool.tile([128, nt], dt.float32, tag="ps")
            for _ in range(40):
                nc.tensor.matmul(
                    out=ps_w[:],
                    lhsT=warm[:, 0:128],
                    rhs=warm[:],
                    start=True,
                    stop=True,
                )

            # ---- dequantize weight slice into 32 per-chunk tiles [128, no] bf16
            wd_tiles = [None] * n_chunks
            qw_sbs = []
            zf_tiles = [None] * kblocks
            sbc_tiles = [None] * kblocks

            def load_block(blk):
                qwl_sb = qw_pool.tile([128, no], dt.int16, tag=f"qwl{blk % 2}")
                qwh_sb = qw_pool.tile([128, no], dt.int16, tag=f"qwh{blk % 2}")
                qw_sbs.append((qwl_sb, qwh_sb))
                zf = zf_pool.tile(
                    [128, no], dt.int16, tag=f"zf{blk % 2}", name=f"zf{blk}"
                )
                nc.sync.dma_start(out=zf[:], in_=zf_d[blk * 128 : (blk + 1) * 128, :])
                zf_tiles[blk] = zf
                sbc = sbc_pool.tile([128, no], dt.float16, tag=f"sbc{blk % 2}")
                nc.sync.dma_start(
                    out=qwl_sb[:], in_=qwl_d[blk * 128 : (blk + 1) * 128, :]
                )
                nc.sync.dma_start(
                    out=qwh_sb[:], in_=qwh_d[blk * 128 : (blk + 1) * 128, :]
                )
                nc.scalar.dma_start(
                    out=sbc[:], in_=sc_d[blk * 128 : (blk + 1) * 128, :]
                )
                sbc_tiles[blk] = sbc

            load_block(0)
            for blk in range(kblocks):
                qwl_sb, qwh_sb = qw_sbs[blk]
                for j in range(8):
                    cc = blk * 8 + j
                    wi = wi_pool.tile([128, no], dt.int16, tag="wi_i")
                    nc.vector.tensor_scalar(
                        out=wi[:],
                        in0=(qwl_sb if j < 4 else qwh_sb)[:],
                        scalar1=4 * (j % 4),
                        scalar2=15,
                        op0=alu.logical_shift_right,
                        op1=alu.bitwise_and,
                    )
                    wb = wi_pool.tile([128, no], dt.bfloat16, tag="wi_b")
                    nc.vector.tensor_sub(out=wb[:], in0=wi[:], in1=zf_tiles[blk][:])
                    wdc = wd_pool.tile([128, no], dt.bfloat16, tag=f"wd{cc}")
                    nc.vector.tensor_mul(out=wdc[:], in0=wb[:], in1=sbc_tiles[blk][:])
                    wd_tiles[cc] = wdc
                    if j == 0 and blk + 1 < kblocks:
                        # prefetch next block's inputs early
                        load_block(blk + 1)

            # ---- stream token tiles: cast-DMA to bf16, matmul, store
            for tt in range(t // nt):
                xbf_t = []
                for k in range(n_chunks):
                    xb = xbf_pool.tile(
                        [128, nt], dt.bfloat16, tag=f"xb{k}", name=f"xb{k}"
                    )
                    nc.gpsimd.dma_start(
                        out=xb[:],
                        in_=xt_d[k * 128 : (k + 1) * 128, tt * nt : (tt + 1) * nt],
                    )
                    xbf_t.append(xb)
                for os_ in range(no // 128):
                    ps = ps_pool.tile([128, nt], dt.float32, tag="ps", name="ps")
                    ks = [(os_ * 4 + i) % n_chunks for i in range(n_chunks)]
                    for i, k in enumerate(ks):
                        nc.tensor.matmul(
                            out=ps[:],
                            lhsT=wd_tiles[k][:, os_ * 128 : (os_ + 1) * 128],
                            rhs=xbf_t[k][:],
                            start=(i == 0),
                            stop=(i == n_chunks - 1),
                        )
                    yo = yo_pool.tile([128, nt], dt.bfloat16, name="yo")
                    if os_ % 2 == 0:
                        nc.scalar.copy(out=yo[:], in_=ps[:])
                    else:
                        nc.vector.tensor_copy(out=yo[:], in_=ps[:])
                    nc.sync.dma_start(
                        out=y_d[os_ * 128 : (os_ + 1) * 128, tt * nt : (tt + 1) * nt],
                        in_=yo[:],
                    )
    nc.compile()
    return nc


def shard_inputs(x, qweight, qzeros, scales, no=NO, t=TP_T):
    """Host-side sharding + the k-interleave layout for x^T."""
    x2 = np.asarray(x, dtype=np.float32).reshape(T_TOTAL, IN_F)
    qweight = np.asarray(qweight)
    qzeros = np.asarray(qzeros)
    scales = np.asarray(scales)

    # xr[blk*1024 + j*128 + p, tok] = x2[tok, blk*1024 + 8p + j]
    xv = x2.reshape(T_TOTAL, IN_F // 1024, 128, 8)  # [tok, blk, p, j]
    xt_shards = []
    for r in range(DP):
        sl = xv[r * t : (r + 1) * t]  # [t, blk, p, j]
        xr = np.ascontiguousarray(sl.transpose(1, 3, 2, 0)).reshape(IN_F, t)
        xt_shards.append(xr)

    qw16 = qweight.view(np.int16).reshape(qweight.shape[0], qweight.shape[1], 2)
    in_maps = []
    for core in range(N_CORES):
        r, c = divmod(core, TP)
        qwc = qw16[:, c * no : (c + 1) * no]
        qzc = qzeros[:, c * (no // 8) : (c + 1) * (no // 8)]
        shifts = (np.arange(8, dtype=np.int32) * 4)[None, None, :]
        zc = ((qzc[:, :, None] >> shifts) & 15).astype(np.int16).reshape(
            qzc.shape[0], no
        )
        in_maps.append(
            {
                "xt": xt_shards[r],
                "qwl": np.ascontiguousarray(qwc[:, :, 0]),
                "qwh": np.ascontiguousarray(qwc[:, :, 1]),
                "zf": np.repeat(zc, 16, axis=0),
                "sc": np.repeat(scales[:, c * no : (c + 1) * no], 16, axis=0),
            }
        )
    return in_maps


def assemble_output(results, no=NO, t=TP_T):
    y = np.empty((T_TOTAL, OUT_F), dtype=np.float32)
    for core in range(N_CORES):
        r, c = divmod(core, TP)
        yp = np.asarray(results[core]["y"])  # [no, t] bf16
        y[r * t : (r + 1) * t, c * no : (c + 1) * no] = yp.T.astype(np.float32)
    return y.reshape(B, S, OUT_F)


_NC_CACHE = {}


def run(x, qweight, qzeros, scales, trace=False, tmpdir=None):
    from concourse.bass_utils import run_bass_kernel_spmd

    if "nc" not in _NC_CACHE:
        _NC_CACHE["nc"] = build_nc()
    nc = _NC_CACHE["nc"]
    in_maps = shard_inputs(x, qweight, qzeros, scales)
    res = run_bass_kernel_spmd(
        nc, in_maps, list(range(N_CORES)), trace=trace, tmpdir=tmpdir
    )
    return assemble_output(res.results), res


def kernel(x, qweight, qzeros, scales):
    y, _ = run(x, qweight, qzeros, scales)
    return y


# revision 31
# speedup vs baseline: 1.1392x; 1.0006x over previous
"""AutoRound/GPTQ int4 linear on 8 Trainium2 NeuronCores.

y = x @ dequant(qweight, qzeros, scales), computed in bf16 like the torch
module: deq = (w_int4 - zeros[g]) * scales[g] in fp32, cast to bf16;
y = bf16_matmul(x.bf16, deq.bf16) with fp32 accumulation, output cast
back to fp32.

Sharding: 8 cores = 4-way tensor-parallel on out_features (1024 each)
x 2-way data-parallel on tokens (4096 each). Each core dequantizes its
weight slice on-chip and computes y_part^T = deq_slice^T-style matmul
producing [1024 out, 4096 tok] bf16; the host reassembles.

Device-side layout tricks:
- The contraction (in_features) index is interleaved so that SBUF
  k-chunk `cc = blk*8 + j` holds k = blk*1024 + 8*p + j at partition p.
  Nibble j of packed qweight row p (of the block's 128 rows) is then
  exactly the weight for partition p of chunk cc, so the int4 unpack is
  one fused shift+mask tensor_scalar per chunk with a *constant* shift.
  The host feeds x^T with rows permuted the same way so the matmul
  contraction stays consistent.
- qweight is split on the host into int16 low/high planes so the whole
  dequant chain runs in 16-bit DVE fast modes: extract at 4x, subtract
  (int16-int16 -> bf16) and scale-multiply (bf16*fp16 -> bf16) at 2x.
- zeros are unpacked and, like scales, group-replicated x16 on the host
  (tiny metadata) so each block needs just three plain 128-partition
  loads instead of many small broadcast DMAs (SP issue is ~0.6us/DMA).
- x is cast fp32 -> bf16 inline by SWDGE (gpsimd) converting DMAs, which
  round-to-nearest-even exactly like the reference's astype(bf16).
- A short dummy-matmul warmup keeps the PE HAM clock-gate at 2.4 GHz
  through the dequant window, and each output group's k-accumulation
  order is rotated so PSUM groups chase the dequant frontier instead of
  all stalling on the last-produced chunk.
"""

import numpy as np
import ml_dtypes

PACK = 8
IN_F = 4096
OUT_F = 4096
GROUP = 128
B, S = 4, 2048
T_TOTAL = B * S  # 8192

N_CORES = 8
TP = 4  # out_feature shards
DP = 2  # token shards
NO = OUT_F // TP  # 1024 out features per core
TP_T = T_TOTAL // DP  # 4096 tokens per core
NT = 512  # token tile (matmul moving free dim / one PSUM bank)
KB = IN_F // 1024  # k blocks of 1024 (8 chunks of 128 each)


def build_nc(no=NO, t=TP_T, nt=NT, kblocks=KB):
    import concourse.bacc as bacc
    import concourse.mybir as mybir
    from concourse.tile import TileContext

    dt = mybir.dt
    alu = mybir.AluOpType
    n_chunks = kblocks * 8

    nc = bacc.Bacc("TRN2", target_bir_lowering=False, debug=False)

    xt_d = nc.dram_tensor("xt", [n_chunks * 128, t], dt.float32, kind="ExternalInput")
    # low/high int16 halves of the packed int32 qweight/qzeros (host-split):
    # nibbles j=0..3 live in the low half, j=4..7 in the high half.
    qwl_d = nc.dram_tensor("qwl", [kblocks * 128, no], dt.int16, kind="ExternalInput")
    qwh_d = nc.dram_tensor("qwh", [kblocks * 128, no], dt.int16, kind="ExternalInput")
    # zeros (host-unpacked int16) and scales, group rows pre-replicated x16
    # on host so row p of a block corresponds to group p//16
    zf_d = nc.dram_tensor("zf", [kblocks * 128, no], dt.int16, kind="ExternalInput")
    sc_d = nc.dram_tensor("sc", [kblocks * 128, no], dt.float16, kind="ExternalInput")
    y_d = nc.dram_tensor("y", [no, t], dt.bfloat16, kind="ExternalOutput")

    with TileContext(nc) as tc:
        with (
            tc.tile_pool(name="wd", bufs=1) as wd_pool,
            tc.tile_pool(name="qw", bufs=2) as qw_pool,
            tc.tile_pool(name="sbc", bufs=2) as sbc_pool,
            tc.tile_pool(name="zf", bufs=2) as zf_pool,
            tc.tile_pool(name="wi", bufs=5) as wi_pool,
            tc.tile_pool(name="xbf", bufs=2) as xbf_pool,
            tc.tile_pool(name="ps", bufs=8, space="PSUM") as ps_pool,
            tc.tile_pool(name="yo", bufs=4) as yo_pool,
        ):
            # ---- PE warm-up: dummy matmuls on a memset tile so the HAM
            # clock-gate reaches 2.4 GHz before the real stream starts.
            warm = qw_pool.tile([128, nt], dt.bfloat16, tag="warm")
            nc.vector.memset(warm[:], 0.0)
            ps_w = ps_pool.tile([128, nt], dt.float32, tag="ps")
            for _ in range(40):
                nc.tensor.matmul(
                    out=ps_w[:],
                    lhsT=warm[:, 0:128],
                    rhs=warm[:],
                    start=True,
                    stop=True,
                )

            # ---- dequantize weight slice into 32 per-chunk tiles [128, no] bf16
            wd_tiles = [None] * n_chunks
            qw_sbs = []
            zf_tiles = [None] * kblocks
            sbc_tiles = [None] * kblocks

            def load_block(blk):
                qwl_sb = qw_pool.tile([128, no], dt.int16, tag=f"qwl{blk % 2}")
                qwh_sb = qw_pool.tile([128, no], dt.int16, tag=f"qwh{blk % 2}")
                qw_sbs.append((qwl_sb, qwh_sb))
                zf = zf_pool.tile(
                    [128, no], dt.int16, tag=f"zf{blk % 2}", name=f"zf{blk}"
                )
                nc.sync.dma_start(out=zf[:], in_=zf_d[blk * 128 : (blk + 1) * 128, :])
                zf_tiles[blk] = zf
                sbc = sbc_pool.tile([128, no], dt.float16, tag=f"sbc{blk % 2}")
                nc.sync.dma_start(
                    out=qwl_sb[:], in_=qwl_d[blk * 128 : (blk + 1) * 128, :]
                )
                nc.sync.dma_start(
                    out=qwh_sb[:], in_=qwh_d[blk * 128 : (blk + 1) * 128, :]
                )
                nc.scalar.dma_start(
                    out=sbc[:], in_=sc_d[blk * 128 : (blk + 1) * 128, :]
                )
                sbc_tiles[blk] = sbc

            load_block(0)
            for blk in range(kblocks):
                qwl_sb, qwh_sb = qw_sbs[blk]
                for j in range(8):
                    cc = blk * 8 + j
                    wi = wi_pool.tile([128, no], dt.int16, tag="wi_i")
                    nc.vector.tensor_scalar(
                        out=wi[:],
                        in0=(qwl_sb if j < 4 else qwh_sb)[:],
                        scalar1=4 * (j % 4),
                        scalar2=15,
                        op0=alu.logical_shift_right,
                        op1=alu.bitwise_and,
                    )
                    wb = wi_pool.tile([128, no], dt.bfloat16, tag="wi_b")
                    nc.vector.tensor_sub(out=wb[:], in0=wi[:], in1=zf_tiles[blk][:])
                    wdc = wd_pool.tile([128, no], dt.bfloat16, tag=f"wd{cc}")
                    nc.vector.tensor_mul(out=wdc[:], in0=wb[:], in1=sbc_tiles[blk][:])
                    wd_tiles[cc] = wdc
                    if j == 0 and blk + 1 < kblocks:
                        # prefetch next block's inputs early
                        load_block(blk + 1)

            # ---- stream token tiles: cast-DMA to bf16, matmul, store
            for tt in range(t // nt):
                xbf_t = []
                for k in range(n_chunks):
                    xb = xbf_pool.tile(
                        [128, nt], dt.bfloat16, tag=f"xb{k}", name=f"xb{k}"
                    )
                    nc.gpsimd.dma_start(
                        out=xb[:],
                        in_=xt_d[k * 128 : (k + 1) * 128, tt * nt : (tt + 1) * nt],
                    )
                    xbf_t.append(xb)
                for os_ in range(no // 128):
                    ps = ps_pool.tile([128, nt], dt.float32, tag="ps", name="ps")
                    ks = [(os_ * 4 + i) % n_chunks for i in range(n_chunks)]
                    for i, k in enumerate(ks):
                        nc.tensor.matmul(
                            out=ps[:],
                            lhsT=wd_tiles[k][:, os_ * 128 : (os_ + 1) * 128],
                            rhs=xbf_t[k][:],
                            start=(i == 0),
                            stop=(i == n_chunks - 1),
                        )
                    yo = yo_pool.tile([128, nt], dt.bfloat16, name="yo")
                    if os_ % 2 == 0:
                        nc.scalar.copy(out=yo[:], in_=ps[:])
                    else:
                        nc.vector.tensor_copy(out=yo[:], in_=ps[:])
                    nc.sync.dma_start(
                        out=y_d[os_ * 128 : (os_ + 1) * 128, tt * nt : (tt + 1) * nt],
                        in_=yo[:],
                    )
    nc.compile()
    return nc


def shard_inputs(x, qweight, qzeros, scales, no=NO, t=TP_T):
    """Host-side sharding + the k-interleave layout for x^T."""
    x2 = np.ascontiguousarray(np.asarray(x, dtype=np.float32).reshape(T_TOTAL, IN_F))
    qweight = np.ascontiguousarray(np.asarray(qweight, dtype=np.int32))
    qzeros = np.ascontiguousarray(np.asarray(qzeros, dtype=np.int32))
    scales = np.ascontiguousarray(np.asarray(scales, dtype=np.float16))

    # xr[blk*1024 + j*128 + p, tok] = x2[tok, blk*1024 + 8p + j]
    xv = x2.reshape(T_TOTAL, IN_F // 1024, 128, 8)  # [tok, blk, p, j]
    xt_shards = []
    for r in range(DP):
        sl = xv[r * t : (r + 1) * t]  # [t, blk, p, j]
        xr = np.ascontiguousarray(sl.transpose(1, 3, 2, 0)).reshape(IN_F, t)
        xt_shards.append(xr)

    qw16 = qweight.view(np.int16).reshape(qweight.shape[0], qweight.shape[1], 2)
    in_maps = []
    for core in range(N_CORES):
        r, c = divmod(core, TP)
        qwc = qw16[:, c * no : (c + 1) * no]
        qzc = qzeros[:, c * (no // 8) : (c + 1) * (no // 8)]
        shifts = (np.arange(8, dtype=np.int32) * 4)[None, None, :]
        zc = ((qzc[:, :, None] >> shifts) & 15).astype(np.int16).reshape(
            qzc.shape[0], no
        )
        in_maps.append(
            {
                "xt": xt_shards[r],
                "qwl": np.ascontiguousarray(qwc[:, :, 0]),
                "qwh": np.ascontiguousarray(qwc[:, :, 1]),
                "zf": np.repeat(zc, 16, axis=0),
                "sc": np.repeat(scales[:, c * no : (c + 1) * no], 16, axis=0),
            }
        )
    return in_maps


def assemble_output(results, no=NO, t=TP_T):
    y = np.empty((T_TOTAL, OUT_F), dtype=np.float32)
    for core in range(N_CORES):
        r, c = divmod(core, TP)
        yp = np.asarray(results[core]["y"])  # [no, t] bf16
        y[r * t : (r + 1) * t, c * no : (c + 1) * no] = yp.T.astype(np.float32)
    return y.reshape(B, S, OUT_F)


_NC_CACHE = {}


def run(x, qweight, qzeros, scales, trace=False, tmpdir=None):
    from concourse.bass_utils import run_bass_kernel_spmd

    if "nc" not in _NC_CACHE:
        _NC_CACHE["nc"] = build_nc()
    nc = _NC_CACHE["nc"]
    in_maps = shard_inputs(x, qweight, qzeros, scales)
    res = run_bass_kernel_spmd(
        nc, in_maps, list(range(N_CORES)), trace=trace, tmpdir=tmpdir
    )
    return assemble_output(res.results), res


def kernel(x, qweight, qzeros, scales):
    y, _ = run(x, qweight, qzeros, scales)
    return y
